# revision 7
# baseline (speedup 1.0000x reference)
"""Bass/Trainium2 kernel for nn_DiscAdvLossForSource_PartialDA.

Computes, over full inputs (B=32768, C=2048):
    prob = softmax(input, axis=1)
    pt   = prob[r, target[r]];  pd = prob[r, -1];  w = class_weight[target[r]]
    loss = sum(w * (-log(pt)*(1-pd) - log(1-pt)*pd)) / B
(with the reference's eps branches at pt==0 / pt==1)

Strategy: pure data parallel over 8 NeuronCores, 4096 rows per core.
Per core the only full-width work per [128, 2048] tile is a row-max
(VectorE) and an exp with accumulate (ScalarE, accum_out gives the row
sum of exp) -- both fit under the 1 MiB/tile DMA time, so the kernel is
HBM-bound.  pt / pd / w are fetched with indirect DMA gathers (flat
element indices precomputed on host from target), and the final per-
sample loss math runs on tiny [128, 32] tiles.  Host sums the 8 per-core
per-sample outputs and divides by B.
"""

import numpy as np
from contextlib import ExitStack

import concourse.bacc as bacc
import concourse.bass as bass
import concourse.tile as tile
from concourse import mybir
from concourse.bass_utils import run_bass_kernel_spmd

N_CORES = 8
B, C = 32768, 2048
BS = B // N_CORES          # rows per core
P = 128                    # partitions
NT = BS // P               # [128, C] tiles per core
EPS = 1e-6

_cache = {}


def build_nc():
    nc = bacc.Bacc("TRN2", target_bir_lowering=False, debug=False,
                   num_devices=N_CORES)
    x = nc.dram_tensor("x", [BS * C], mybir.dt.float32, kind="ExternalInput")
    gidx = nc.dram_tensor("gidx", [P, NT], mybir.dt.int32,
                          kind="ExternalInput")
    tgt = nc.dram_tensor("tgt", [P, NT], mybir.dt.int32, kind="ExternalInput")
    cw = nc.dram_tensor("cw", [C], mybir.dt.float32, kind="ExternalInput")
    out = nc.dram_tensor("out", [P, NT], mybir.dt.float32,
                         kind="ExternalOutput")

    f32 = mybir.dt.float32
    with ExitStack() as ctx:
        tc = ctx.enter_context(tile.TileContext(nc))
        xpool = ctx.enter_context(tc.tile_pool(name="xp", bufs=4))
        epool = ctx.enter_context(tc.tile_pool(name="ep", bufs=2))
        sp = ctx.enter_context(tc.tile_pool(name="sp", bufs=1))

        gidx_t = sp.tile([P, NT], mybir.dt.int32)
        tgt_t = sp.tile([P, NT], mybir.dt.int32)
        xt_g = sp.tile([P, NT], f32)
        xd_g = sp.tile([P, NT], f32)
        w = sp.tile([P, NT], f32)
        mneg = sp.tile([P, NT], f32)
        z = sp.tile([P, NT], f32)

        nc.sync.dma_start(gidx_t[:], gidx.ap())
        nc.sync.dma_start(tgt_t[:], tgt.ap())

        # Gather x[r, target[r]] and class_weight[target[r]].  HW indirect
        # DMA consumes exactly one offset per partition per instruction, so
        # issue one gather per [128]-row column.
        x_2d = x.ap().rearrange("(n one) -> n one", one=1)
        cw_2d = cw.ap().rearrange("(n one) -> n one", one=1)
        for j in range(NT):
            nc.gpsimd.indirect_dma_start(
                out=xt_g[:, j:j + 1], out_offset=None, in_=x_2d,
                in_offset=bass.IndirectOffsetOnAxis(ap=gidx_t[:, j:j + 1],
                                                    axis=0))
            nc.gpsimd.indirect_dma_start(
                out=w[:, j:j + 1], out_offset=None, in_=cw_2d,
                in_offset=bass.IndirectOffsetOnAxis(ap=tgt_t[:, j:j + 1],
                                                    axis=0))
        # x[r, C-1] has static indices: one strided DMA.
        x_lastcol = x.ap().rearrange("(i p c) -> p i c", p=P, c=C)[:, :, C - 1]
        nc.sync.dma_start(xd_g[:], x_lastcol)

        # Main streaming loop: row max and sum(exp(x - max)) per tile.
        x3 = x.ap().rearrange("(n p c) -> n p c", p=P, c=C)
        for i in range(NT):
            xt_tile = xpool.tile([P, C], f32, tag="xt")
            nc.sync.dma_start(xt_tile[:], x3[i])
            nc.vector.reduce_max(out=mneg[:, i:i + 1], in_=xt_tile[:],
                                 axis=mybir.AxisListType.X, negate=True)
            e_scr = epool.tile([P, C], f32, tag="e")
            nc.scalar.activation(e_scr[:], xt_tile[:],
                                 mybir.ActivationFunctionType.Exp,
                                 bias=mneg[:, i:i + 1], scale=1.0,
                                 accum_out=z[:, i:i + 1])

        # Epilogue on [P, NT] tiles.
        sh_t = sp.tile([P, NT], f32)
        sh_d = sp.tile([P, NT], f32)
        et = sp.tile([P, NT], f32)
        ed = sp.tile([P, NT], f32)
        zr = sp.tile([P, NT], f32)
        pt = sp.tile([P, NT], f32)
        pd = sp.tile([P, NT], f32)
        t0 = sp.tile([P, NT], f32)
        t1 = sp.tile([P, NT], f32)
        log_pt = sp.tile([P, NT], f32)
        log_1mpt = sp.tile([P, NT], f32)
        per = sp.tile([P, NT], f32)

        A = mybir.AluOpType
        nc.vector.tensor_add(sh_t[:], xt_g[:], mneg[:])
        nc.vector.tensor_add(sh_d[:], xd_g[:], mneg[:])
        nc.scalar.activation(et[:], sh_t[:], mybir.ActivationFunctionType.Exp)
        nc.scalar.activation(ed[:], sh_d[:], mybir.ActivationFunctionType.Exp)
        nc.vector.reciprocal(zr[:], z[:])
        nc.vector.tensor_mul(pt[:], et[:], zr[:])
        nc.vector.tensor_mul(pd[:], ed[:], zr[:])

        # log_pt = log(pt + EPS*(pt==0))
        nc.vector.tensor_scalar(out=t0[:], in0=pt[:], scalar1=0.0,
                                scalar2=EPS, op0=A.is_equal, op1=A.mult)
        nc.vector.tensor_add(t0[:], t0[:], pt[:])
        nc.scalar.activation(log_pt[:], t0[:],
                             mybir.ActivationFunctionType.Ln)
        # log_1mpt = log(1 - pt*(1 - EPS*(pt==1)))
        nc.vector.tensor_scalar(out=t1[:], in0=pt[:], scalar1=1.0,
                                scalar2=-EPS, op0=A.is_equal, op1=A.mult)
        nc.vector.tensor_scalar(out=t1[:], in0=t1[:], scalar1=1.0,
                                scalar2=None, op0=A.add)
        nc.vector.tensor_mul(t1[:], t1[:], pt[:])
        nc.vector.tensor_scalar(out=t1[:], in0=t1[:], scalar1=-1.0,
                                scalar2=1.0, op0=A.mult, op1=A.add)
        nc.scalar.activation(log_1mpt[:], t1[:],
                             mybir.ActivationFunctionType.Ln)

        # per = w * (log_pt*(pd-1) - log_1mpt*pd)
        nc.vector.tensor_scalar(out=t0[:], in0=pd[:], scalar1=-1.0,
                                scalar2=None, op0=A.add)
        nc.vector.tensor_mul(t0[:], log_pt[:], t0[:])
        nc.vector.tensor_mul(t1[:], log_1mpt[:], pd[:])
        nc.vector.tensor_sub(t0[:], t0[:], t1[:])
        nc.vector.tensor_mul(per[:], t0[:], w[:])

        nc.sync.dma_start(out.ap(), per[:])

    nc.compile()
    return nc


def prepare_in_maps(input, target, class_weight):
    x = np.ascontiguousarray(np.asarray(input, dtype=np.float32))
    t = np.asarray(target).astype(np.int32)
    cw = np.ascontiguousarray(np.asarray(class_weight, dtype=np.float32))
    p = np.arange(P, dtype=np.int64)[:, None]
    i = np.arange(NT, dtype=np.int64)[None, :]
    r = i * P + p                                    # [P, NT] row-in-shard
    in_maps = []
    for c in range(N_CORES):
        ts = t[c * BS:(c + 1) * BS]
        tgt_cols = ts[r]                             # [P, NT]
        gidx = (r * C + tgt_cols).astype(np.int32)
        in_maps.append({
            "x": x[c * BS:(c + 1) * BS].reshape(-1),
            "gidx": gidx,
            "tgt": tgt_cols.astype(np.int32),
            "cw": cw,
        })
    return in_maps


def kernel(input, target, class_weight, _trace=False, **_run_kwargs):
    if "nc" not in _cache:
        _cache["nc"] = build_nc()
    nc = _cache["nc"]
    in_maps = prepare_in_maps(input, target, class_weight)
    res = run_bass_kernel_spmd(nc, in_maps, core_ids=list(range(N_CORES)),
                               trace=_trace, **_run_kwargs)
    _cache["last_results"] = res
    tot = sum(r["out"].astype(np.float64).sum() for r in res.results)
    return np.float32(tot / B)
